# revision 13
# baseline (speedup 1.0000x reference)
"""Trainium2 Bass kernel for nn_ConfidanceLoss.

reference semantics (see harness reference):
  occ   = (batchVolume == 1)                       [B, 32, 32, 32]
  pooled= 5x5x5 windowed max (zero-pad, stride 1)
  sub   = pooled sampled at cell centers 2,6,..,30 -> [B, 8, 8, 8] (x, y, z)
  iou   = transpose to (z, y, x) then flatten      -> [B, 512], j = z*64 + y*8 + x
  returns (confi [B,512] f32, iou [B,512] f32, in_use [B,512] i32)

Window for center 4i+2 is [4i, 4i+4] clipped to 31, so per axis:
  out[i] = max(V[4i], V[4i+1], V[4i+2], V[4i+3], V[4i+4 if 4i+4<=31])

Separable max-pool, 128 batch items per core on the 128 SBUF partitions
(8 cores x 128 = B=1024); all reductions run along the free dimension.
Pass order is chosen for DVE read contiguity (inner-strided reads run
~3x slower than contiguous runs):
  1. pool A2 (middle axis): reads contiguous 32-elem a3 rows
  2. pool A1 (outer axis):  incremental per chunk, contiguous 256-elem planes
  3. pool A3 (inner axis):  stride-4 reads but on 16x-reduced data
The volume streams in as A1-plane chunks (first two small so DVE starts
early) on the sync (SP) HWDGE ring; confi passthrough + most outputs use
the scalar (ACT) ring so they never block volume loads.
"""

import sys

for _p in ("/opt/trn_rl_repo",):
    if _p not in sys.path:
        sys.path.insert(0, _p)

import numpy as np

import concourse.bass as bass  # noqa: F401  (registers types)
import concourse.tile as tile
from concourse import bacc, mybir
from concourse.bass_utils import run_bass_kernel_spmd

B = 1024
GRID = 32
P = 512
N_CORES = 8
ITEMS = B // N_CORES  # 128 batch items per core == 128 partitions
VOL = GRID * GRID * GRID  # 32768
ROW = GRID * GRID  # elems per A1-plane per item
CHUNK_PLANES = [2, 2] + [4] * 7  # sums to 32

_I32 = mybir.dt.int32
_F32 = mybir.dt.float32
_BF16 = mybir.dt.bfloat16


def _build():
    nc = bacc.Bacc(
        "TRN2",
        target_bir_lowering=False,
        debug=False,
        num_devices=N_CORES,
    )
    vol = nc.dram_tensor("batchVolume", [ITEMS, VOL], _I32, kind="ExternalInput")
    confi = nc.dram_tensor("confi", [ITEMS, P], _F32, kind="ExternalInput")
    out_confi = nc.dram_tensor("out_confi", [ITEMS, P], _F32, kind="ExternalOutput")
    out_iou = nc.dram_tensor("out_iou", [ITEMS, P], _F32, kind="ExternalOutput")
    out_inuse = nc.dram_tensor("out_inuse", [ITEMS, P], _I32, kind="ExternalOutput")

    with tile.TileContext(nc) as tc:
        with (
            tc.tile_pool(name="vol", bufs=8) as vol_pool,
            tc.tile_pool(name="tmp", bufs=2) as tmp_pool,
            tc.tile_pool(name="misc", bufs=1) as misc_pool,
        ):
            # confi passthrough on the ACT ring (tiny, off the volume path)
            cbuf = misc_pool.tile([ITEMS, P], _F32, tag="cbuf")
            nc.scalar.dma_start(cbuf[:], confi.ap())
            nc.scalar.dma_start(out_confi.ap(), cbuf[:])

            # after A2-pool: I [a1=32, c2=8, a3=32] per item
            I = misc_pool.tile([ITEMS, GRID * 8 * GRID], _BF16, tag="interm")
            # after A1-pool: Pp [c1=8, c2=8, a3=32]
            Pp = misc_pool.tile([ITEMS, 8 * 8 * GRID], _BF16, tag="ppool")
            PpV = Pp[:].rearrange("p (c1 f) -> p c1 f", c1=8, f=256)
            PQ = Pp[:].rearrange("p (c1 c2 a3) -> p c1 c2 a3", c1=8, c2=8, a3=GRID)

            # A3-pool + output writes for a half (c1 range [w0, w1))
            s1 = misc_pool.tile([ITEMS, P], _BF16, tag="s1")
            s2 = misc_pool.tile([ITEMS, P], _BF16, tag="s2")
            S1 = s1[:].rearrange("p (c1 c2 c3) -> p c1 c2 c3", c1=8, c2=8, c3=8)
            S2 = s2[:].rearrange("p (c1 c2 c3) -> p c1 c2 c3", c1=8, c2=8, c3=8)
            iou_sb = misc_pool.tile([ITEMS, P], _F32, tag="iou")
            inuse_sb = misc_pool.tile([ITEMS, P], _I32, tag="inuse")
            # S* hold [c1=x, c2=y, c3=z]; out j = z*64 + y*8 + x
            PV = iou_sb[:].rearrange("p (c3 c2 c1) -> p c1 c2 c3", c1=8, c2=8, c3=8)

            def pass_a3(w0, w1):
                q = PQ[:, w0:w1]
                a1, b1 = S1[:, w0:w1], S2[:, w0:w1]
                nc.vector.tensor_max(a1, q[:, :, :, 0::4], q[:, :, :, 1::4])
                nc.vector.tensor_max(a1[:, :, :, 0:7], a1[:, :, :, 0:7], q[:, :, :, 4::4])
                nc.vector.tensor_max(b1, q[:, :, :, 2::4], q[:, :, :, 3::4])
                nc.vector.tensor_max(PV[:, w0:w1], a1, b1)

            plane0 = 0
            n_chunks = len(CHUNK_PLANES)
            for c, planes in enumerate(CHUNK_PLANES):
                n = planes * ROW
                off = plane0 * ROW
                vc = vol_pool.tile([ITEMS, n], _I32, tag="vc")
                nc.sync.dma_start(vc[:], vol.ap()[:, off : off + n])
                V = vc[:].rearrange(
                    "p (a1 a2 a3) -> p a1 a2 a3", a1=planes, a2=GRID, a3=GRID
                )
                # ---- pass 1: pool A2 -> I planes [plane0, plane0+planes)
                tn = planes * 8 * GRID
                tB = tmp_pool.tile([ITEMS, tn], _BF16, tag="tB")
                Bv = tB[:].rearrange(
                    "p (a1 c2 a3) -> p a1 c2 a3", a1=planes, c2=8, a3=GRID
                )
                Ic = I[:, 256 * plane0 : 256 * (plane0 + planes)]
                A = Ic.rearrange(
                    "p (a1 c2 a3) -> p a1 c2 a3", a1=planes, c2=8, a3=GRID
                )
                nc.vector.tensor_max(A, V[:, :, 0::4, :], V[:, :, 1::4, :])
                nc.vector.tensor_max(
                    A[:, :, 0:7, :], A[:, :, 0:7, :], V[:, :, 4::4, :]
                )
                nc.vector.tensor_max(Bv, V[:, :, 2::4, :], V[:, :, 3::4, :])
                nc.vector.tensor_max(Ic, Ic, tB[:])  # in-place flat combine
                IcV = Ic.rearrange("p (a1 f) -> p a1 f", a1=planes, f=256)

                # ---- pass 2 (incremental): fold these planes into A1 windows
                if c == 0:  # planes 0,1 -> start window 0
                    nc.vector.tensor_max(PpV[:, 0:1, :], IcV[:, 0:1, :], IcV[:, 1:2, :])
                elif c == 1:  # planes 2,3 -> finish window 0 (sans 5th plane)
                    nc.vector.tensor_max(
                        PpV[:, 0:1, :], PpV[:, 0:1, :], IcV[:, 0:1, :]
                    )
                    nc.vector.tensor_max(
                        PpV[:, 0:1, :], PpV[:, 0:1, :], IcV[:, 1:2, :]
                    )
                elif planes == 4:  # planes [4w, 4w+3] for window w
                    w = plane0 // 4
                    m = tmp_pool.tile([ITEMS, 2 * 256], _BF16, tag="m")
                    mV = m[:].rearrange("p (h f) -> p h f", h=2, f=256)
                    nc.vector.tensor_max(mV, IcV[:, 0::2, :], IcV[:, 1::2, :])
                    nc.vector.tensor_max(
                        PpV[:, w : w + 1, :], mV[:, 0:1, :], mV[:, 1:2, :]
                    )
                    # this chunk's first plane (4w) is window w-1's 5th plane
                    nc.vector.tensor_max(
                        PpV[:, w - 1 : w, :], PpV[:, w - 1 : w, :], IcV[:, 0:1, :]
                    )
                    if w == 4:  # windows 0..3 final -> first A3 half
                        pass_a3(0, 4)
                elif plane0 % 4 == 0:  # first half of a split window w
                    w = plane0 // 4
                    nc.vector.tensor_max(
                        PpV[:, w : w + 1, :], IcV[:, 0:1, :], IcV[:, 1:2, :]
                    )
                    # this chunk's first plane (4w) is window w-1's 5th plane
                    nc.vector.tensor_max(
                        PpV[:, w - 1 : w, :], PpV[:, w - 1 : w, :], IcV[:, 0:1, :]
                    )
                else:  # second half of a split window w
                    w = plane0 // 4
                    nc.vector.tensor_max(
                        PpV[:, w : w + 1, :], PpV[:, w : w + 1, :], IcV[:, 0:1, :]
                    )
                    nc.vector.tensor_max(
                        PpV[:, w : w + 1, :], PpV[:, w : w + 1, :], IcV[:, 1:2, :]
                    )
                plane0 += planes

            pass_a3(4, 8)
            nc.vector.tensor_copy(inuse_sb[:], iou_sb[:])

            nc.sync.dma_start(out_iou.ap(), iou_sb[:])
            nc.scalar.dma_start(out_inuse.ap(), inuse_sb[:])

    nc.compile()
    return nc


_NC_CACHE = None


def _get_nc():
    global _NC_CACHE
    if _NC_CACHE is None:
        _NC_CACHE = _build()
    return _NC_CACHE


def _make_in_maps(confi_rlt, batchVolume):
    confi = np.ascontiguousarray(
        np.asarray(confi_rlt).reshape(B, P).astype(np.float32, copy=False)
    )
    vol = np.ascontiguousarray(
        np.asarray(batchVolume).reshape(B, VOL).astype(np.int32, copy=False)
    )
    in_maps = []
    for c in range(N_CORES):
        sl = slice(ITEMS * c, ITEMS * (c + 1))
        in_maps.append(
            {
                "batchVolume": np.ascontiguousarray(vol[sl]),
                "confi": np.ascontiguousarray(confi[sl]),
            }
        )
    return in_maps


def _run(confi_rlt, batchVolume, trace=False, **spmd_kwargs):
    nc = _get_nc()
    res = run_bass_kernel_spmd(
        nc,
        _make_in_maps(confi_rlt, batchVolume),
        core_ids=list(range(N_CORES)),
        trace=trace,
        **spmd_kwargs,
    )
    confi_full = np.concatenate([r["out_confi"] for r in res.results], axis=0)
    iou_full = np.concatenate([r["out_iou"] for r in res.results], axis=0)
    inuse_full = np.concatenate([r["out_inuse"] for r in res.results], axis=0)
    return (confi_full, iou_full, inuse_full), res


def kernel(shape_rlt, trans_rlt, quat_rlt, confi_rlt, batchVolume):
    out, _ = _run(confi_rlt, batchVolume)
    return out
